# revision 1
# baseline (speedup 1.0000x reference)
"""Trainium2 Bass kernel for: out = relu(einsum('bcs,cs->bs', x, w) + bias).

Full shapes: x [32, 2048, 4096] f32, w [2048, 4096] f32, bias [4096] f32.
Sharding: the s-axis (4096) is split across 8 cores (512 each). Each core
reads its x slice (128 MiB) and w/bias slice (4 MiB) once — the minimum
possible HBM traffic — and produces out[:, s_slice]. Gather = concat.

The stream is DMA-bound. Measured engine behavior (packet traces): the
16 DMA engines move 32 KiB descriptors at ~27 B/ns each (~432 GB/s
aggregate) when fed from both hardware DGE queues, but a single queue is
serial per trigger — transfer plus ~2.6 us of turnaround — and paces at
~340 GB/s. In the good steady state the two queues' packets interleave
1:1 on every engine at full per-packet speed. Design:

  - p-major channel layout: channel c maps to (partition, k) = (c//16,
    c%16), so each partition's per-batch slice is one contiguous 32 KiB
    DRAM run -> 128 big descriptors per batch instead of 2048 x 2 KiB.
  - dual-queue alternation: even batches trigger on the sync ring, odd
    batches on the scalar ring, so the engines always have descriptors
    from both queues and stream at the fabric limit.
  - output drains ride the gpsimd (SWDGE) queue and the relu for batch
    b is emitted after batch b+2's trigger, so neither ring's DGE ever
    sits behind a compute wait.
  - bf16 products: DVE multiplies x*w writing bf16, so the PE's
    128-partition ones-matmul reduction runs at 1 cyc/row instead of 4
    (also halves PE power draw, which matters: activity throttling
    caps the DMA fabric when the compute engines run hot). Rounding is
    ~2^-9 per product; l2 rel err ~2e-3 vs the f32 reference.
  - the final batch streams in eighths (alternating queues) so the
    post-stream chain (last mul -> 2 matmuls -> relu -> drain) is short.

Per-core dataflow (partitions = channel/16, free = k*512 + s):
  DMA   x[b] slice  -> SBUF [128, 8192]             (4 MiB per batch)
  DVE   prod = xb * w  (f32 mul, bf16 write)
  PE    ones-matmul per k-block accumulating the 128-partition reduction
        of each [128, 512] block into PSUM [1, 512]; the bias row is
        folded in as a K=1 f32 matmul that opens the accumulation group.
  ACT   relu during PSUM -> SBUF copy; GPSIMD drains 2 KiB to out[b].
"""

import numpy as np

B, C, S_FULL = 32, 2048, 4096
N_CORES = 8
S = S_FULL // N_CORES          # 512 s-values per core
P = 128                        # SBUF partitions
CB = C // P                    # 16 channel blocks per partition
FREE = CB * S                  # 8192 f32 per partition per batch

_nc_cache = {}


def _build():
    import concourse.bacc as bacc
    import concourse.mybir as mybir
    import concourse.tile as tile

    f32 = mybir.dt.float32
    bf16 = mybir.dt.bfloat16
    nc = bacc.Bacc(
        "TRN2",
        target_bir_lowering=False,
        debug=False,
        enable_asserts=False,
        num_devices=N_CORES,
    )

    x = nc.dram_tensor("xs", [B, C, S], f32, kind="ExternalInput").ap()
    w = nc.dram_tensor("ws", [C, S], f32, kind="ExternalInput").ap()
    bias = nc.dram_tensor("bs", [1, S], f32, kind="ExternalInput").ap()
    out = nc.dram_tensor("out", [B, S], f32, kind="ExternalOutput").ap()

    with tile.TileContext(nc) as tc:
        with (
            tc.tile_pool(name="const", bufs=1) as cpool,
            tc.tile_pool(name="xp", bufs=4) as xpool,
            tc.tile_pool(name="pp", bufs=2) as ppool,
            tc.tile_pool(name="ps", bufs=6, space="PSUM") as pspool,
            tc.tile_pool(name="op", bufs=2) as opool,
        ):
            # w leads the sync ring; the first x half starts concurrently
            # on the scalar ring behind the bias load.
            w_sb = cpool.tile([P, FREE], f32)
            nc.sync.dma_start(w_sb[:], w.rearrange("(p k) s -> p (k s)", p=P))

            ones_f32 = cpool.tile([P, 1], f32)
            nc.vector.memset(ones_f32[:], 1.0)
            ones_bf = cpool.tile([P, 1], bf16)
            nc.vector.tensor_copy(ones_bf[:], ones_f32[:])

            bias_sb = cpool.tile([1, S], f32)
            nc.scalar.dma_start(bias_sb[:], bias[:])

            x_r = x.rearrange("b (p k) s -> b p (k s)", p=P)
            pending = []  # (b, ps, ob) awaiting relu+drain emission

            def flush_one():
                pb, pps, pob = pending.pop(0)
                nc.scalar.activation(
                    pob[:], pps[:], mybir.ActivationFunctionType.Relu
                )
                # 2 KiB drain on the gpsimd SWDGE queue: keeps both HW
                # rings' DGEs free of drain turnarounds.
                nc.gpsimd.dma_start(out[pb : pb + 1], pob[:])

            for b in range(B):
                ring = nc.sync if b % 2 == 0 else nc.scalar
                xb = xpool.tile([P, FREE], f32, tag="xb")
                prod = ppool.tile([P, FREE], bf16, tag="prod")
                if b == B - 1:
                    nchunk = 8
                elif b == B - 2:
                    nchunk = 2
                else:
                    nchunk = 1
                CH = CB // nchunk
                ps = pspool.tile([1, S], f32)
                # bias fold-in: K=1 matmul opens the accumulation group
                nc.tensor.matmul(
                    ps[:], ones_f32[0:1, 0:1], bias_sb[:], start=True, stop=False
                )
                for h in range(nchunk):
                    r0 = h * CH * S
                    r1 = (h + 1) * CH * S
                    cring = ring if nchunk == 1 else (
                        nc.sync if h % 2 == 0 else nc.scalar
                    )
                    cring.dma_start(xb[:, r0:r1], x_r[b, :, r0:r1])
                    nc.vector.tensor_mul(
                        prod[:, r0:r1], xb[:, r0:r1], w_sb[:, r0:r1]
                    )
                    last = h == nchunk - 1
                    for i in range(CH):
                        j = h * CH + i
                        nc.tensor.matmul(
                            ps[:],
                            ones_bf[:],
                            prod[:, j * S : (j + 1) * S],
                            start=False,
                            stop=(last and i == CH - 1),
                        )

                ob = opool.tile([1, S], f32, tag="ob")
                pending.append((b, ps, ob))
                # defer relu/drain 2 batches so the scalar ring's next x
                # trigger is never queued behind a wait-on-PE
                if len(pending) > 2:
                    flush_one()
            while pending:
                flush_one()

    nc.compile()
    return nc


def _get_nc():
    if "nc" not in _nc_cache:
        _nc_cache["nc"] = _build()
    return _nc_cache["nc"]


def _shard_inputs(x, weights, bias):
    x = np.asarray(x)
    weights = np.asarray(weights)
    bias = np.asarray(bias)
    in_maps = []
    for i in range(N_CORES):
        sl = slice(i * S, (i + 1) * S)
        in_maps.append(
            {
                "xs": np.ascontiguousarray(x[:, :, sl], dtype=np.float32),
                "ws": np.ascontiguousarray(weights[:, sl], dtype=np.float32),
                "bs": np.ascontiguousarray(
                    bias[sl].reshape(1, S), dtype=np.float32
                ),
            }
        )
    return in_maps


def _run(inputs, trace=False, trace_cores=None):
    from concourse import bass_utils

    nc = _get_nc()
    in_maps = _shard_inputs(inputs["x"], inputs["weights"], inputs["bias"])
    res = bass_utils.run_bass_kernel_spmd(
        nc,
        in_maps,
        core_ids=list(range(N_CORES)),
        trace=trace,
        trace_cores=trace_cores,
    )
    out = np.concatenate([r["out"] for r in res.results], axis=1)
    return out, res


def kernel(x, weights, bias):
    out, _ = _run({"x": x, "weights": weights, "bias": bias})
    return out

